# revision 1
# baseline (speedup 1.0000x reference)
"""Trainium2 Bass kernel for nn_CombinedHiddenEncoder (5-layer GCN stack on a
fixed random graph, N=50000 nodes, E=600000 edges + self loops).

Algebraic restructure (S = D^-1/2 (A+I) D^-1/2 is shared by all 5 GCNConvs and
commutes with right-multiplication by the weight matrices):

    U      = feature @ (W1 @ W3[:HD]) + condition @ (W2 @ W3[HD:])
    V      = S^3 @ U
    mean   = V @ Wm + s2*(c1@Wm) + s1*(b3@Wm) + bm      (s1 = S@1, s2 = S@s1)
    logvar = V @ Wv + s2*(c1@Wv) + s1*(b3@Wv) + bv
    z      = noise * exp(0.5*logvar) + mean

with c1 = b1@W3[:HD] + b2@W3[HD:].  Writing T = diag(1/sqrt(deg)) and A01 for
the 0/1 adjacency (incl. self loops), S^3 = T A01 T^2 A01 T^2 A01 T, so on
device every sparse step is an *unweighted* gather + one-hot matmul
accumulation, with cheap per-node diagonal scalings in between.

Distribution: nodes are sharded across the 8 cores (6250 each, padded to
6272 = 49*128).  Each round every core holds a replicated [50176, 128] table
of the current X (AllGather), gathers the source rows for the edges whose
destination lives on it (dst-sorted, 128-edge chunks), and scatter-adds via
  psum[dstloc, feat] += onehot(dstloc).T @ gathered
TensorEngine matmuls accumulating in PSUM.  The one-hot matrices are built on
the fly by the vector engine (iota == dstloc).  Because dma_gather indices are
int16 (max 32767 < 50176 table rows), edges are split by source-row parity and
gathered with a stride-2 access pattern (idx = row >> 1).

Host-side work is limited to: weight folding (tiny), integer edge
bookkeeping/sharding, and the O(E) degree/s1/s2 vectors needed to build the
shards; all O(N*D) math runs on the NeuronCores.
"""

import numpy as np

import concourse.bass as bass
import concourse.mybir as mybir
import concourse.tile as tile
from concourse import bacc
from concourse.bass_utils import run_bass_kernel_spmd
from concourse.masks import make_identity

F32 = mybir.dt.float32
I16 = mybir.dt.int16

# ---- problem constants (hardcoded per contest contract) ----
N, E = 50000, 600000
FD, CD, HD, LD = 256, 128, 128, 64
CORES = 8
SHARD = N // CORES            # 6250
TILES = (SHARD + 127) // 128  # 49
R = TILES * 128               # 6272 padded rows per core
TR = CORES * R                # 50176 table rows
GROUP = 7                     # dst-tiles per gather call
NGROUPS = TILES // GROUP      # 7

_prog_cache: dict = {}


# --------------------------------------------------------------------------
# Bass program builder
# --------------------------------------------------------------------------
def build_program(nc_par: int, variant: str = "full"):
    """One SPMD program (identical on all 8 cores); nc_par = chunks of 128
    gather slots per (dst-tile, parity) group.

    variant: experiment knob — "full", "nogather" (skip dma_gather),
    "nomm" (skip one-hot+matmul), "nocc" (skip collectives)."""
    do_gather = variant not in ("nogather",)
    do_mm = variant not in ("nomm",)
    do_cc = variant not in ("nocc",)
    n_rounds = int(variant[1:]) if variant.startswith("r") else 3
    nc = bacc.Bacc(None, target_bir_lowering=False)

    chunks_per_call = GROUP * nc_par            # chunks per dma_gather call
    idxs_per_call = chunks_per_call * 128
    idxcols_per_call = idxs_per_call // 16
    ncalls = NGROUPS * 2
    total_chunks = ncalls * chunks_per_call     # = TILES*2*nc_par

    # ---- I/O ----
    xfT = nc.dram_tensor("xfT", [FD, R], F32, kind="ExternalInput")
    xcT = nc.dram_tensor("xcT", [CD, R], F32, kind="ExternalInput")
    noise_in = nc.dram_tensor("noise_in", [R, LD], F32, kind="ExternalInput")
    aw = nc.dram_tensor("aw", [FD, HD], F32, kind="ExternalInput")
    bw = nc.dram_tensor("bw", [CD, HD], F32, kind="ExternalInput")
    wm = nc.dram_tensor("wm", [HD, LD], F32, kind="ExternalInput")
    wv = nc.dram_tensor("wv", [HD, LD], F32, kind="ExternalInput")
    cm = nc.dram_tensor("cm", [4, LD], F32, kind="ExternalInput")
    cv = nc.dram_tensor("cv", [4, LD], F32, kind="ExternalInput")
    dinv_p = nc.dram_tensor("dinv_p", [128, TILES], F32, kind="ExternalInput")
    dinv2_p = nc.dram_tensor("dinv2_p", [128, TILES], F32, kind="ExternalInput")
    srows = nc.dram_tensor("srows", [4, R], F32, kind="ExternalInput")
    idx_all = nc.dram_tensor(
        "idx_all", [128, ncalls * idxcols_per_call], I16, kind="ExternalInput")
    dstloc_all = nc.dram_tensor(
        "dstloc_all", [128, total_chunks], F32, kind="ExternalInput")

    z_out = nc.dram_tensor("z_out", [R, LD], F32, kind="ExternalOutput")
    mean_out = nc.dram_tensor("mean_out", [R, LD], F32, kind="ExternalOutput")
    logvar_out = nc.dram_tensor("logvar_out", [R, LD], F32, kind="ExternalOutput")

    # ---- internal DRAM ----
    bounce = [nc.dram_tensor(f"xb{r}", [R, HD], F32) for r in range(3)]
    tabs = [nc.dram_tensor(f"tab{r}", [TR, HD], F32, addr_space="Shared")
            for r in range(3)]
    rg = [list(range(CORES))]

    with tile.TileContext(nc) as tc:
        with tc.tile_pool(name="const", bufs=1) as cpool:
            # constants / metadata resident in SBUF
            ident = cpool.tile([128, 128], F32)
            make_identity(nc, ident[:])
            colidx = cpool.tile([128, 128], F32)
            nc.gpsimd.iota(colidx[:], pattern=[[1, 128]], base=0,
                           channel_multiplier=0,
                           allow_small_or_imprecise_dtypes=True)
            a0_s = cpool.tile([128, HD], F32)
            a1_s = cpool.tile([128, HD], F32)
            b_s = cpool.tile([128, HD], F32)
            nc.sync.dma_start(out=a0_s[:], in_=aw[0:128, :])
            nc.sync.dma_start(out=a1_s[:], in_=aw[128:256, :])
            nc.sync.dma_start(out=b_s[:], in_=bw[:, :])
            wm_s = cpool.tile([128, LD], F32)
            wv_s = cpool.tile([128, LD], F32)
            nc.sync.dma_start(out=wm_s[:], in_=wm[:, :])
            nc.sync.dma_start(out=wv_s[:], in_=wv[:, :])
            cm_s = cpool.tile([4, LD], F32)
            cv_s = cpool.tile([4, LD], F32)
            nc.sync.dma_start(out=cm_s[:4, :], in_=cm[:, :])
            nc.sync.dma_start(out=cv_s[:4, :], in_=cv[:, :])
            dinv_s = cpool.tile([128, TILES], F32)
            dinv2_s = cpool.tile([128, TILES], F32)
            nc.sync.dma_start(out=dinv_s[:], in_=dinv_p[:, :])
            nc.sync.dma_start(out=dinv2_s[:], in_=dinv2_p[:, :])
            idx_s = cpool.tile([128, ncalls * idxcols_per_call], I16)
            nc.sync.dma_start(out=idx_s[:], in_=idx_all[:, :])
            dloc_s = cpool.tile([128, total_chunks], F32)
            nc.sync.dma_start(out=dloc_s[:], in_=dstloc_all[:, :])

            with tc.tile_pool(name="psum", bufs=4, space="PSUM") as mmpool, \
                 tc.tile_pool(name="hpsum", bufs=4, space="PSUM") as hpool:

                # ---------------- stage 0: X0 = T (Xf A + Xc B) --------------
                with tc.tile_pool(name="s0", bufs=1) as s0pool:
                    xf0_s = s0pool.tile([128, R], F32)
                    xf1_s = s0pool.tile([128, R], F32)
                    xc_s = s0pool.tile([128, R], F32)
                    x0strip = s0pool.tile([128, R], F32)
                    nc.sync.dma_start(out=xf0_s[:], in_=xfT[0:128, :])
                    nc.sync.dma_start(out=xf1_s[:], in_=xfT[128:256, :])
                    nc.sync.dma_start(out=xc_s[:], in_=xcT[:, :])
                    for t in range(TILES):
                        cs = slice(t * 128, (t + 1) * 128)
                        ps = mmpool.tile([128, HD], F32, name=f"s0ps{t}",
                                         tag="mm")
                        nc.tensor.matmul(ps[:], xf0_s[:, cs], a0_s[:],
                                         start=True, stop=False)
                        nc.tensor.matmul(ps[:], xf1_s[:, cs], a1_s[:],
                                         start=False, stop=False)
                        nc.tensor.matmul(ps[:], xc_s[:, cs], b_s[:],
                                         start=False, stop=True)
                        nc.vector.tensor_scalar(
                            out=x0strip[:, cs], in0=ps[:],
                            scalar1=dinv_s[:, t:t + 1], scalar2=None,
                            op0=mybir.AluOpType.mult)
                    nc.sync.dma_start(
                        out=bounce[0].ap().rearrange("(t p) h -> p t h", p=128),
                        in_=x0strip[:].rearrange("p (t h) -> p t h", h=HD))

                if do_cc:
                    nc.gpsimd.collective_compute(
                        "AllGather", mybir.AluOpType.bypass, replica_groups=rg,
                        ins=[bounce[0].ap()], outs=[tabs[0].ap()])

                # ---------------- 3 sparse rounds ---------------------------
                with tc.tile_pool(name="rnd", bufs=1) as rpool, \
                     tc.tile_pool(name="gpool", bufs=4) as gpool, \
                     tc.tile_pool(name="qpool", bufs=6) as qpool, \
                     tc.tile_pool(name="hd", bufs=3) as hdpool:
                    xstrip = rpool.tile([128, R], F32)
                    for rnd in range(n_rounds):
                        tab = tabs[rnd]
                        tab2 = tab.ap().rearrange("(r two) h -> r two h", two=2)
                        last = rnd == n_rounds - 1
                        for g in range(NGROUPS):
                            gts = []
                            for par in (0, 1):
                                gt = gpool.tile([128, chunks_per_call, 128],
                                                F32, name=f"gt{rnd}_{g}_{par}",
                                                tag="gath")
                                call = g * 2 + par
                                icols = slice(call * idxcols_per_call,
                                              (call + 1) * idxcols_per_call)
                                if not do_gather:
                                    # keep the tile written so Tile allocates it
                                    nc.vector.tensor_scalar(
                                        out=gt[:, 0, :], in0=colidx[:],
                                        scalar1=1.0, scalar2=None,
                                        op0=mybir.AluOpType.mult)
                                if do_gather:
                                    nc.gpsimd.dma_gather(
                                        out_ap=gt[:],
                                        in_ap=tab2[:, par, :],
                                        idxs_ap=idx_s[:, icols],
                                        num_idxs=idxs_per_call,
                                        num_idxs_reg=idxs_per_call,
                                        elem_size=HD,
                                        elem_step=2 * HD,
                                        single_packet=False)
                                gts.append(gt)
                            for ti in range(GROUP):
                                t = g * GROUP + ti
                                ps = mmpool.tile([128, HD], F32,
                                                 name=f"ps{rnd}_{t}", tag="mm")
                                nmm = 2 * nc_par if do_mm else 1
                                k = 0
                                for par in ((0, 1) if do_mm else (0,)):
                                    for c in range(nc_par if do_mm else 1):
                                        col = ((g * 2 + par) * GROUP + ti) \
                                            * nc_par + c
                                        q = qpool.tile([128, 128], F32,
                                                       name=f"q{rnd}_{t}_{k}",
                                                       tag="q")
                                        nc.vector.tensor_scalar(
                                            out=q[:], in0=colidx[:],
                                            scalar1=dloc_s[:, col:col + 1],
                                            scalar2=None,
                                            op0=mybir.AluOpType.is_equal)
                                        nc.tensor.matmul(
                                            ps[:], q[:],
                                            gts[par][:, ti * nc_par + c, :],
                                            start=(k == 0),
                                            stop=(k == nmm - 1))
                                        k += 1
                                cs = slice(t * 128, (t + 1) * 128)
                                if not last:
                                    # X_{r+1} tile = dinv^2 * psum
                                    nc.vector.tensor_scalar(
                                        out=xstrip[:, cs], in0=ps[:],
                                        scalar1=dinv2_s[:, t:t + 1],
                                        scalar2=None,
                                        op0=mybir.AluOpType.mult)
                                else:
                                    # V tile = dinv * psum, then the head
                                    v = hdpool.tile([128, HD], F32,
                                                    name=f"v{t}", tag="v")
                                    nc.vector.tensor_scalar(
                                        out=v[:], in0=ps[:],
                                        scalar1=dinv_s[:, t:t + 1],
                                        scalar2=None,
                                        op0=mybir.AluOpType.mult)
                                    pst = mmpool.tile([128, HD], F32,
                                                      name=f"pst{t}", tag="mm")
                                    nc.tensor.transpose(pst[:], v[:], ident[:])
                                    vT = hdpool.tile([128, HD], F32,
                                                     name=f"vT{t}", tag="vT")
                                    nc.vector.tensor_copy(out=vT[:], in_=pst[:])
                                    sr = hdpool.tile([4, 128], F32,
                                                     name=f"sr{t}", tag="sr")
                                    nc.sync.dma_start(out=sr[:4, :],
                                                      in_=srows[:, cs])
                                    nz = hdpool.tile([128, LD], F32,
                                                     name=f"nz{t}", tag="nz")
                                    nc.sync.dma_start(out=nz[:],
                                                      in_=noise_in[cs, :])
                                    mps = hpool.tile([128, LD], F32,
                                                     name=f"mps{t}", tag="hp")
                                    nc.tensor.matmul(mps[:], vT[:], wm_s[:],
                                                     start=True, stop=False)
                                    nc.tensor.matmul(mps[:], sr[:3, :],
                                                     cm_s[:3, :],
                                                     start=False, stop=True)
                                    lps = hpool.tile([128, LD], F32,
                                                     name=f"lps{t}", tag="hp")
                                    nc.tensor.matmul(lps[:], vT[:], wv_s[:],
                                                     start=True, stop=False)
                                    nc.tensor.matmul(lps[:], sr[:3, :],
                                                     cv_s[:3, :],
                                                     start=False, stop=True)
                                    mn = hdpool.tile([128, LD], F32,
                                                     name=f"mn{t}", tag="mn")
                                    lv = hdpool.tile([128, LD], F32,
                                                     name=f"lv{t}", tag="lv")
                                    ex = hdpool.tile([128, LD], F32,
                                                     name=f"ex{t}", tag="ex")
                                    zt = hdpool.tile([128, LD], F32,
                                                     name=f"zt{t}", tag="zt")
                                    nc.vector.tensor_copy(out=mn[:], in_=mps[:])
                                    nc.vector.tensor_copy(out=lv[:], in_=lps[:])
                                    nc.scalar.activation(
                                        out=ex[:], in_=lps[:],
                                        func=mybir.ActivationFunctionType.Exp,
                                        scale=0.5)
                                    nc.vector.tensor_tensor(
                                        out=zt[:], in0=nz[:], in1=ex[:],
                                        op=mybir.AluOpType.mult)
                                    nc.vector.tensor_tensor(
                                        out=zt[:], in0=zt[:], in1=mn[:],
                                        op=mybir.AluOpType.add)
                                    nc.sync.dma_start(out=z_out[cs, :],
                                                      in_=zt[:])
                                    nc.sync.dma_start(out=mean_out[cs, :],
                                                      in_=mn[:])
                                    nc.sync.dma_start(out=logvar_out[cs, :],
                                                      in_=lv[:])
                        if not last:
                            nc.sync.dma_start(
                                out=bounce[rnd + 1].ap().rearrange(
                                    "(t p) h -> p t h", p=128),
                                in_=xstrip[:].rearrange(
                                    "p (t h) -> p t h", h=HD))
                            if do_cc:
                                nc.gpsimd.collective_compute(
                                    "AllGather", mybir.AluOpType.bypass,
                                    replica_groups=rg,
                                    ins=[bounce[rnd + 1].ap()],
                                    outs=[tabs[rnd + 1].ap()])
    nc.finalize()
    return nc


# --------------------------------------------------------------------------
# Host-side preprocessing
# --------------------------------------------------------------------------
def preprocess(feature, condition, edge_index, noise,
               W1, b1, W2, b2, W3, b3, Wm, bm, Wv, bv):
    feature = np.asarray(feature, np.float32)
    condition = np.asarray(condition, np.float32)
    noise = np.asarray(noise, np.float32)
    ei = np.asarray(edge_index).astype(np.int64)
    W1 = np.asarray(W1, np.float32); b1 = np.asarray(b1, np.float32)
    W2 = np.asarray(W2, np.float32); b2 = np.asarray(b2, np.float32)
    W3 = np.asarray(W3, np.float32); b3 = np.asarray(b3, np.float32)
    Wm = np.asarray(Wm, np.float32); bm = np.asarray(bm, np.float32)
    Wv = np.asarray(Wv, np.float32); bv = np.asarray(bv, np.float32)

    loop = np.arange(N, dtype=np.int64)
    src = np.concatenate([ei[0], loop])
    dst = np.concatenate([ei[1], loop])
    deg = np.bincount(dst, minlength=N).astype(np.float64)
    dinv = 1.0 / np.sqrt(deg)
    w = dinv[src] * dinv[dst]
    s1 = np.bincount(dst, weights=w, minlength=N)
    s2 = np.bincount(dst, weights=w * s1[src], minlength=N)
    dinv32 = dinv.astype(np.float32)

    W3a, W3b = W3[:HD], W3[HD:]
    A_w = W1 @ W3a
    B_w = W2 @ W3b
    c1 = b1 @ W3a + b2 @ W3b
    Cm = np.zeros((4, LD), np.float32)
    Cm[:3] = np.stack([c1 @ Wm, b3 @ Wm, bm])
    Cv = np.zeros((4, LD), np.float32)
    Cv[:3] = np.stack([c1 @ Wv, b3 @ Wv, bv])

    node = np.arange(N, dtype=np.int64)
    pos_of_node = (node // SHARD) * R + (node % SHARD)
    pos_src = pos_of_node[src]

    core = dst // SHARD
    d_loc = dst - core * SHARD
    tl = d_loc // 128
    dstloc = d_loc % 128
    parity = pos_src & 1
    idx16 = (pos_src >> 1).astype(np.int64)

    # group key: (core, tile, parity); stable sort then slot assignment
    gid = (core * TILES + tl) * 2 + parity
    ngroups_tot = CORES * TILES * 2
    counts = np.bincount(gid, minlength=ngroups_tot)
    nc_par = int((counts.max() + 127) // 128)
    slots_pp = nc_par * 128

    order = np.argsort(gid, kind="stable")
    gs = gid[order]
    within = np.arange(len(gs)) - np.repeat(
        np.concatenate([[0], np.cumsum(counts)[:-1]]), counts)
    slot = gs * slots_pp + within

    idx_slots = np.zeros(ngroups_tot * slots_pp, np.int16)
    dl_slots = np.full(ngroups_tot * slots_pp, -1.0, np.float32)
    idx_slots[slot] = idx16[order].astype(np.int16)
    dl_slots[slot] = dstloc[order].astype(np.float32)

    # [CORES, TILES, 2, nc_par, 128]
    idx_slots = idx_slots.reshape(CORES, TILES, 2, nc_par, 128)
    dl_slots = dl_slots.reshape(CORES, TILES, 2, nc_par, 128)

    chunks_per_call = GROUP * nc_par
    idxs_per_call = chunks_per_call * 128

    in_maps = []
    for k in range(CORES):
        rows = slice(k * SHARD, (k + 1) * SHARD)
        xfT = np.zeros((FD, R), np.float32)
        xfT[:, :SHARD] = feature[rows].T
        xcT = np.zeros((CD, R), np.float32)
        xcT[:, :SHARD] = condition[rows].T
        nz = np.zeros((R, LD), np.float32)
        nz[:SHARD] = noise[rows]
        dv = np.zeros((TILES, 128), np.float32)
        dv.reshape(-1)[:SHARD] = dinv32[rows]
        sr = np.zeros((4, R), np.float32)
        di = dinv[rows.start:rows.stop]
        sr[0, :SHARD] = (s2[rows] / di).astype(np.float32)
        sr[1, :SHARD] = (s1[rows] / di).astype(np.float32)
        sr[2, :SHARD] = (1.0 / di).astype(np.float32)

        # gather calls: order (g, par); call covers tiles g*GROUP..+GROUP-1
        idx_core = idx_slots[k].reshape(NGROUPS, GROUP, 2, nc_par * 128)
        idx_calls = np.transpose(idx_core, (0, 2, 1, 3)).reshape(
            NGROUPS * 2, idxs_per_call)
        # wrap each call's idx list into 16 partitions, replicate x8
        ic = idx_calls.reshape(NGROUPS * 2, idxs_per_call // 16, 16)
        ic = np.transpose(ic, (2, 0, 1)).reshape(16, -1)
        idx_arr = np.tile(ic, (8, 1))

        dl_core = dl_slots[k].reshape(NGROUPS, GROUP, 2, nc_par, 128)
        # chunk col order: ((g*2+par)*GROUP + ti)*nc_par + c
        dl_cols = np.transpose(dl_core, (0, 2, 1, 3, 4)).reshape(-1, 128).T
        dl_arr = np.ascontiguousarray(dl_cols)

        in_maps.append({
            "xfT": xfT, "xcT": xcT, "noise_in": nz,
            "aw": A_w, "bw": B_w, "wm": Wm, "wv": Wv, "cm": Cm, "cv": Cv,
            "dinv_p": np.ascontiguousarray(dv.T),
            "dinv2_p": np.ascontiguousarray((dv ** 2).T),
            "srows": sr,
            "idx_all": np.ascontiguousarray(idx_arr),
            "dstloc_all": dl_arr,
        })
    return nc_par, in_maps


def kernel(feature, condition, edge_index, noise,
           W1, b1, W2, b2, W3, b3, Wm, bm, Wv, bv, _trace=False):
    nc_par, in_maps = preprocess(feature, condition, edge_index, noise,
                                 W1, b1, W2, b2, W3, b3, Wm, bm, Wv, bv)
    if nc_par not in _prog_cache:
        _prog_cache[nc_par] = build_program(nc_par)
    nc = _prog_cache[nc_par]
    res = run_bass_kernel_spmd(nc, in_maps, list(range(CORES)), trace=_trace)
    z = np.concatenate([res.results[k]["z_out"][:SHARD] for k in range(CORES)])
    mean = np.concatenate(
        [res.results[k]["mean_out"][:SHARD] for k in range(CORES)])
    logvar = np.concatenate(
        [res.results[k]["logvar_out"][:SHARD] for k in range(CORES)])
    if _trace:
        kernel._last_exec_time_ns = res.exec_time_ns
        kernel._last_results = res
    return (z, mean, logvar)

